# revision 1
# baseline (speedup 1.0000x reference)
"""ContrastiveLoss Trainium2 kernel.

Math (matches the jax reference):
    an = l2norm(inputs_col); bn = l2norm(inputs_row)
    sim = an @ bn.T                                     [n, n]
    same = targets_col[:,None] == target_row[None,:]
    pos = same & (sim < 1-1e-5);  neg = ~same & (sim > 0.5)
    loss = sum(where(any(pos,1), sum(pos*(1-sim) + neg*sim, 1), 0)) / n

Distribution: rows of inputs_col are sharded across 8 cores (1024 each);
inputs_row / target_row replicated. Each core emits one fp32 partial sum;
the host adds the 8 partials and divides by n.

Per-core pipeline:
  1. normalize rows in fp32 (ACT square+accum -> sqrt -> DVE reciprocal),
     scale+cast to bf16 (ACT copy with per-partition scale)
  2. PE-transpose A(shard) and B tiles so the contraction dim d lands on
     partitions; B is processed in 2048-column "quarters" to bound SBUF
  3. sim block [128, 1024] = 16 accumulating bf16 matmuls into PSUM
  4. fused elementwise+row-reduce directly on the PSUM block:
       f_pos = relu(-sim + (1-1e-5))          (ACT, reads PSUM)
       gsim  = (sim > 0.5)*sim, rowsum -> g   (DVE scalar_tensor_tensor)
       s     = (t_row_bcast == t_col_p)       (DVE tensor_scalar, bf16)
       q     = s*f_pos, rowsum -> q           (DVE tensor_tensor_reduce)
       sg    = s*gsim,  rowsum -> sg          (DVE tensor_tensor_reduce)
     row_loss = (q > 0) * (q + g - sg)   [since sum_j (1-s)*gsim = g - sg]
  5. partition-sum of row losses via a [128,1]x[128,1] fp32 matmul.

The relu trick: f_pos = relu((1-1e-5) - sim) equals the exact
(1-sim)*[sim < 1-1e-5] up to a -1e-5 bias on passing elements (relative
error ~1e-5 on the final loss) and is exactly zero iff the mask is zero,
so the q>0 "has_pos" gate stays exact.
"""

import numpy as np
from contextlib import ExitStack

import concourse.bass as bass
import concourse.mybir as mybir
import concourse.tile as tile
from concourse import bacc
from concourse.bass import ds, ts
from concourse.masks import make_identity

N = 8192            # rows of inputs_col / inputs_row
D = 1024            # feature dim
NCORES = 8
ROWS = N // NCORES  # inputs_col rows per core
P = 128             # SBUF partitions
NCH = ROWS // P     # i-chunks per core (8)
KT = D // P         # contraction tiles (8)
QJ = 2048           # B-column quarter width
NQ = N // QJ        # quarters (4)
JB = 1024           # elementwise block width (2 PSUM banks)
NJB_Q = QJ // JB    # blocks per quarter (2)
NJB = N // JB       # global blocks (8)

EPS_NORM = 1e-12
EPS_POS = 1e-5
MARGIN = 0.5

F32 = mybir.dt.float32
BF16 = mybir.dt.bfloat16
AF = mybir.ActivationFunctionType
OP = mybir.AluOpType


def _normalize_tile(nc, pools, x_f32, x_bf, eps_tile):
    """x_bf = bf16(x_f32 / sqrt(sum(x_f32^2, axis=1) + EPS_NORM))."""
    small, junk = pools
    sq = small.tile([P, 1], F32, tag="sq")
    sqj = junk.tile([P, D], BF16, tag="sqj")
    nc.scalar.activation(sqj, x_f32, AF.Square, accum_out=sq)
    nc.scalar.activation(sq, sq, AF.Sqrt, bias=eps_tile)
    inv = small.tile([P, 1], F32, tag="inv")
    nc.vector.reciprocal(inv, sq)
    nc.scalar.activation(x_bf, x_f32, AF.Copy, bias=0.0, scale=inv)


def build_kernel_body(tc, out_ap, a_ap, b_ap, tcol_ap, trow_ap):
    nc = tc.nc
    ctx = ExitStack()
    with ctx:
        singles = ctx.enter_context(tc.tile_pool(name="singles", bufs=1))
        small = ctx.enter_context(tc.tile_pool(name="small", bufs=6))
        junk = ctx.enter_context(tc.tile_pool(name="junk", bufs=4))
        stage_f32 = ctx.enter_context(tc.tile_pool(name="stage_f32", bufs=3))
        stage_bf = ctx.enter_context(tc.tile_pool(name="stage_bf", bufs=6))
        btq_pool = ctx.enter_context(tc.tile_pool(name="btq", bufs=2))
        ew_pool = ctx.enter_context(tc.tile_pool(name="ew", bufs=3))
        psum_mm = ctx.enter_context(
            tc.tile_pool(name="psum_mm", bufs=3, space=bass.MemorySpace.PSUM)
        )
        psum_fin = ctx.enter_context(
            tc.tile_pool(name="psum_fin", bufs=1, space=bass.MemorySpace.PSUM)
        )

        ident = singles.tile([P, P], BF16)
        make_identity(nc, ident)
        ones_col = singles.tile([P, 1], F32)
        nc.vector.memset(ones_col, 1.0)
        eps_tile = singles.tile([P, 1], F32)
        nc.vector.memset(eps_tile, EPS_NORM)
        cpos_tile = singles.tile([P, 1], F32)
        nc.vector.memset(cpos_tile, 1.0 - EPS_POS)

        # target_row broadcast to all partitions: [128, N] fp32
        trow_bc = singles.tile([P, N], F32)
        trow_b = bass.AP(
            tensor=trow_ap.tensor,
            offset=trow_ap.offset,
            ap=[[0, P]] + list(trow_ap.ap),
        )
        nc.sync.dma_start(out=trow_bc, in_=trow_b)

        # per-chunk targets_col as per-partition scalars: [128, NCH]
        tcol_sb = singles.tile([P, NCH], F32)
        tcol2 = tcol_ap.rearrange("(c p) -> c p", p=P)
        for c in range(NCH):
            nc.sync.dma_start(out=tcol_sb[:, c : c + 1], in_=tcol2[c][:, None])

        # row-reduction strips, one column per (chunk, jb) block
        rq_strip = singles.tile([P, NCH * NJB], F32)
        rg_strip = singles.tile([P, NCH * NJB], F32)
        rsg_strip = singles.tile([P, NCH * NJB], F32)

        # ---- A shard: normalize + transpose -> AT [128d x (KT, ROWS)] bf16
        at_sb = singles.tile([P, KT, ROWS], BF16)
        for c in range(NCH):
            xf = stage_f32.tile([P, D], F32, tag="xf")
            nc.sync.dma_start(out=xf, in_=a_ap[ds(c * P, P), :])
            xb = stage_bf.tile([P, D], BF16, tag="xb")
            _normalize_tile(nc, (small, junk), xf, xb, eps_tile)
            for k in range(KT):
                nc.sync.dma_start_transpose(
                    out=at_sb[:, k, ds(c * P, P)], in_=xb[:, ds(k * P, P)]
                )

        # ---- main loop over B quarters
        for q in range(NQ):
            bt = btq_pool.tile([P, KT, QJ], BF16, tag="bt")
            for t in range(QJ // P):  # 16 tiles per quarter
                row0 = q * QJ + t * P
                xf = stage_f32.tile([P, D], F32, tag="xf")
                nc.sync.dma_start(out=xf, in_=b_ap[ds(row0, P), :])
                xb = stage_bf.tile([P, D], BF16, tag="xb")
                _normalize_tile(nc, (small, junk), xf, xb, eps_tile)
                for k in range(KT):
                    nc.sync.dma_start_transpose(
                        out=bt[:, k, ds(t * P, P)], in_=xb[:, ds(k * P, P)]
                    )

            for c in range(NCH):
                for jb in range(NJB_Q):
                    jbg = q * NJB_Q + jb
                    col = c * NJB + jbg  # strip column for this block
                    ps = psum_mm.tile([P, JB], F32, tag="ps")
                    for h in range(JB // 512):
                        for k in range(KT):
                            nc.tensor.matmul(
                                ps[:, ds(h * 512, 512)],
                                at_sb[:, k, ds(c * P, P)],
                                bt[:, k, ds(jb * JB + h * 512, 512)],
                                start=(k == 0),
                                stop=(k == KT - 1),
                            )
                    # sim in bf16 SBUF (sole PSUM reader; frees the bank)
                    smb = ew_pool.tile([P, JB], BF16, tag="smb")
                    nc.scalar.activation(smb, ps, AF.Copy, bias=0.0, scale=1.0)
                    # nfpos = min(sim - (1-eps), 0) = -relu((1-eps) - sim)
                    nfpos = ew_pool.tile([P, JB], BF16, tag="nfpos")
                    nc.gpsimd.tensor_scalar(
                        out=nfpos,
                        in0=smb,
                        scalar1=(1.0 - EPS_POS),
                        scalar2=0.0,
                        op0=OP.subtract,
                        op1=OP.min,
                    )
                    # gsim = (sim > margin) * sim ; rowsum -> rg
                    gsim = ew_pool.tile([P, JB], BF16, tag="gsim")
                    nc.vector.scalar_tensor_tensor(
                        out=gsim,
                        in0=smb,
                        scalar=MARGIN,
                        in1=smb,
                        op0=OP.is_gt,
                        op1=OP.mult,
                        accum_out=rg_strip[:, col : col + 1],
                    )
                    # s = (t_row == t_col[p])
                    s = ew_pool.tile([P, JB], BF16, tag="s")
                    nc.gpsimd.tensor_scalar(
                        out=s,
                        in0=trow_bc[:, ds(jbg * JB, JB)],
                        scalar1=tcol_sb[:, c : c + 1],
                        scalar2=None,
                        op0=OP.is_equal,
                    )
                    # q = sum_j s * relu(c - sim) = sum_j (-s) * nfpos -> rq
                    j1 = junk.tile([P, JB], BF16, tag="j1")
                    nc.vector.scalar_tensor_tensor(
                        out=j1,
                        in0=s,
                        scalar=-1.0,
                        in1=nfpos,
                        op0=OP.mult,
                        op1=OP.mult,
                        accum_out=rq_strip[:, col : col + 1],
                    )
                    # sg = sum_j s * gsim -> rsg
                    j2 = junk.tile([P, JB], BF16, tag="j2")
                    nc.vector.scalar_tensor_tensor(
                        out=j2,
                        in0=s,
                        scalar=1.0,
                        in1=gsim,
                        op0=OP.mult,
                        op1=OP.mult,
                        accum_out=rsg_strip[:, col : col + 1],
                    )

        # ---- finalize: row_loss = (rq > 0) * (rq + rg - rsg); sum all rows
        loss_acc = singles.tile([P, 1], F32)
        nc.vector.memset(loss_acc, 0.0)
        for c in range(NCH):
            sl = ds(c * NJB, NJB)
            rq = small.tile([P, 1], F32, tag="rq")
            nc.vector.tensor_reduce(rq, rq_strip[:, sl], axis=mybir.AxisListType.X, op=OP.add)
            rg = small.tile([P, 1], F32, tag="rg")
            nc.vector.tensor_reduce(rg, rg_strip[:, sl], axis=mybir.AxisListType.X, op=OP.add)
            rsg = small.tile([P, 1], F32, tag="rsg")
            nc.vector.tensor_reduce(rsg, rsg_strip[:, sl], axis=mybir.AxisListType.X, op=OP.add)
            ind = small.tile([P, 1], F32, tag="ind")
            nc.vector.tensor_scalar(
                out=ind, in0=rq, scalar1=0.0, scalar2=None, op0=OP.is_gt
            )
            tmp = small.tile([P, 1], F32, tag="tmp")
            nc.vector.tensor_sub(tmp, rg, rsg)
            nc.vector.tensor_add(tmp, tmp, rq)
            nc.vector.tensor_mul(tmp, tmp, ind)
            nc.vector.tensor_add(loss_acc, loss_acc, tmp)

        pfin = psum_fin.tile([1, 1], F32)
        nc.tensor.matmul(pfin, loss_acc, ones_col, start=True, stop=True)
        ob = small.tile([1, 1], F32, tag="ob")
        nc.vector.tensor_copy(ob, pfin)
        nc.sync.dma_start(out=out_ap, in_=ob)


_NC_CACHE = {}


def build_nc(reps=1):
    """reps>1 wraps the body in a hardware For_i loop — used only for
    differential wall-clock timing; the graded path uses reps=1."""
    if reps in _NC_CACHE:
        return _NC_CACHE[reps]
    nc = bacc.Bacc("TRN2", target_bir_lowering=False, debug=False)
    a_ap = nc.dram_tensor("a_shard", [ROWS, D], F32, kind="ExternalInput").ap()
    b_ap = nc.dram_tensor("b_full", [N, D], F32, kind="ExternalInput").ap()
    tcol_ap = nc.dram_tensor("t_col", [ROWS], F32, kind="ExternalInput").ap()
    trow_ap = nc.dram_tensor("t_row", [N], F32, kind="ExternalInput").ap()
    out_ap = nc.dram_tensor("partial", [1, 1], F32, kind="ExternalOutput").ap()
    with tile.TileContext(nc) as tc:
        if reps == 1:
            build_kernel_body(tc, out_ap, a_ap, b_ap, tcol_ap, trow_ap)
        else:
            with tc.For_i(0, reps, 1):
                build_kernel_body(tc, out_ap, a_ap, b_ap, tcol_ap, trow_ap)
    nc.compile()
    _NC_CACHE[reps] = nc
    return nc


def make_in_maps(inputs_col, targets_col, inputs_row, target_row):
    b_full = np.ascontiguousarray(np.asarray(inputs_row, dtype=np.float32))
    trow = np.asarray(target_row).astype(np.float32)
    in_maps = []
    for c in range(NCORES):
        sl = slice(c * ROWS, (c + 1) * ROWS)
        in_maps.append(
            {
                "a_shard": np.ascontiguousarray(
                    np.asarray(inputs_col[sl], dtype=np.float32)
                ),
                "b_full": b_full,
                "t_col": np.asarray(targets_col[sl]).astype(np.float32),
                "t_row": trow,
            }
        )
    return in_maps


def kernel(**inputs):
    from concourse.bass_utils import run_bass_kernel_spmd

    nc = build_nc()
    in_maps = make_in_maps(
        inputs["inputs_col"],
        inputs["targets_col"],
        inputs["inputs_row"],
        inputs["target_row"],
    )
    res = run_bass_kernel_spmd(nc, in_maps, list(range(NCORES))).results
    total = 0.0
    for c in range(NCORES):
        total += float(res[c]["partial"][0, 0])
    return np.float32(total / N)

